# revision 13
# baseline (speedup 1.0000x reference)
"""Trainium2 Bass kernel for nn_ProjectionLayer: mean-pool + projection +
L2-normalize + cosine-sim matrix / pairwise-distance denominator.

Reference math (fp32):
    g = mean(features, axis=2) @ W.T + bias        # [b, out_c]
    g = g / max(||g||_row, 1e-12)                  # L2 normalize rows
    sim = g @ g.T                                  # [b, b]
    dist = ||g + 1e-6||_row                        # [b]
    out = sim / max(dist_i, dist_j, 1e-8)

Sharding: data-parallel over batch (64 rows per core, 8 cores). The
normalized rows are transposed on-chip, cast to bf16 and AllGather'd
together with a reciprocal-distance column (a [128, 257] payload,
66 KB/rank), so every core forms its [64, 512] block of the output with
no post-gather transposes: the sim matmul streams its rhs straight out
of the gathered buffer via a strided access pattern.
"""

import sys

if "/opt/trn_rl_repo" not in sys.path:
    sys.path.insert(0, "/opt/trn_rl_repo")

import numpy as np

# Problem shapes (hardcoded per contract)
B_FULL = 512     # batch
C_IN = 2048      # in channels (contraction dim of projection)
T_POOL = 196     # pooled (time) dim
O_OUT = 512      # out channels
N_CORES = 8

PD_EPS = 1e-6
NORM_EPS = 1e-12
DENO_EPS = 1e-8


def build_kernel(b_full, c_in, t_pool, o_out, n_cores, bg=2, cpp=8):
    """Emit the Bass module (SPMD program, identical on every core).

    cpp = channels per partition in the feature-pooling layout: partition p of
    chunk k holds channels c = span*k + cpp*p + j (j in [0, cpp)), which makes
    each DMA descriptor a cpp*t_pool*4-byte contiguous run (6.3 KB at cpp=8 —
    big descriptors keep the 16 SDMA engines at line rate). The channel
    interleave is undone for free by building W^T chunks from stride-cpp
    column slices of W.
    """
    import concourse.mybir as mybir
    import concourse.tile as tile
    from concourse import bacc
    from concourse.masks import make_identity

    f32 = mybir.dt.float32
    bf16 = mybir.dt.bfloat16
    AL = mybir.AluOpType
    AF = mybir.ActivationFunctionType

    bc = b_full // n_cores          # batch rows per core
    span = 128 * cpp                # channels per chunk
    nk = c_in // span               # channel chunks
    nbg = bc // bg                  # batch groups per chunk
    oc = o_out // 128               # out-channel chunks (for transposes)
    hb = 0                          # set below: batch-half split for projection
    agc = oc * bc + 1               # AG payload cols: 4*64 gnT cols + rdl
    hb = bc // 2
    assert bc % bg == 0 and c_in % span == 0 and o_out % 128 == 0

    nc = bacc.Bacc("TRN2", target_bir_lowering=False, debug=False,
                   enable_asserts=False, num_devices=n_cores)
    feat = nc.dram_tensor("features", [bc, c_in, t_pool], f32,
                          kind="ExternalInput").ap()
    w_in = nc.dram_tensor("w", [o_out, c_in], f32, kind="ExternalInput").ap()
    bias_in = nc.dram_tensor("bias", [1, o_out], f32, kind="ExternalInput").ap()
    out_d = nc.dram_tensor("out", [bc, b_full], f32, kind="ExternalOutput").ap()

    with tile.TileContext(nc) as tc:
        with (
            tc.tile_pool(name="const", bufs=1) as constp,
            tc.tile_pool(name="wload", bufs=1) as wlp,
            tc.tile_pool(name="wtp", bufs=1) as wtp,
            tc.tile_pool(name="featp", bufs=6) as fp,
            tc.tile_pool(name="lhsp", bufs=1) as lp,
            tc.tile_pool(name="postp", bufs=1) as pp,
            tc.tile_pool(name="psrot", bufs=2, space="PSUM") as psp,
            tc.tile_pool(name="psfix", bufs=1, space="PSUM") as psgp,
            tc.tile_pool(name="dram", bufs=1, space="DRAM") as dp,
        ):
            # ---- constants ----
            ident = constp.tile([128, 128], f32, name="ident")
            make_identity(nc, ident)
            identh = constp.tile([128, 128], bf16, name="identh")
            nc.vector.tensor_copy(identh[:], ident[:])
            ones = constp.tile([1, bc], f32, name="ones")
            nc.vector.memset(ones, 1.0)
            bias_sb = constp.tile([1, o_out], f32, name="bias_sb")
            nc.scalar.dma_start(bias_sb[:], bias_in[:])
            warm = constp.tile([1, 1], f32, name="warm")
            nc.vector.memset(warm, 1.0)
            nc.scalar.sqrt(warm[:], warm[:])
            # AG payload staging: gnT [128, 4*64] bf16 | rdl col (zero pad)
            ag_sb = pp.tile([128, agc], bf16, name="ag_sb")
            nc.gpsimd.memset(ag_sb, 0.0)

            p4 = [lp.tile([128, bc, cpp], f32, name=f"p4_{k}") for k in range(nk)]
            wl = []
            wt4 = [None] * (nk * cpp)
            gps = psgp.tile([bc, o_out], f32, name="gps")

            # ---- pooling (k outer -> incremental projection), fused with
            # W load / transpose mid-stream so both HWDGE rings start on
            # feature tiles immediately.
            idma = 0
            for k in range(nk):
                for ibg in range(nbg):
                    ft = fp.tile([128, bg, cpp, t_pool], f32, name="ft")
                    src = feat[ibg * bg:(ibg + 1) * bg,
                               k * span:(k + 1) * span, :].rearrange(
                                   "b (p j) t -> p b j t", j=cpp)
                    dma_eng = nc.sync if idma % 2 == 0 else nc.scalar
                    dma_eng.dma_start(ft[:], src)
                    nc.vector.reduce_sum(p4[k][:, ibg * bg:(ibg + 1) * bg, :],
                                         ft[:], axis=mybir.AxisListType.X)
                    idma += 1

                    if k == 0 and ibg == 3:
                        # W loads: 4 MB total, interleaved into the ring queues
                        for l in range(oc):
                            wli = wlp.tile([128, c_in], f32, name=f"wl{l}")
                            eng = nc.sync if l % 2 == 0 else nc.scalar
                            eng.dma_start(wli[:], w_in[l * 128:(l + 1) * 128, :])
                            wl.append(wli)
                    if k == 0 and ibg == 8:
                        # W^T / t_pool, interleave-matched layout.
                        # wt4[kk*cpp+j] rows: partition p <-> channel
                        # c = span*kk + cpp*p + j. PE transposes (idle engine),
                        # PSUM->SBUF scale-copies on DVE (slack under DMA).
                        for kk in range(nk):
                            for j in range(cpp):
                                pswt = psp.tile([128, o_out], f32, name="pswt",
                                                tag="rot")
                                for l in range(oc):
                                    srcw = wl[l][:, kk * span:(kk + 1) * span
                                                 ].rearrange(
                                        "o (p j) -> o p j", j=cpp)[:, :, j]
                                    nc.tensor.transpose(
                                        pswt[:, l * 128:(l + 1) * 128],
                                        srcw, ident[:])
                                wtk = wtp.tile([128, o_out], bf16,
                                               name=f"wt{kk}_{j}")
                                nc.scalar.mul(wtk[:], pswt[:], 1.0 / t_pool)
                                wt4[kk * cpp + j] = wtk

                # incremental projection for this channel chunk: cast the
                # pooled block to bf16 (one DVE copy), single-pass bf16 matmuls
                p4h = lp.tile([128, bc, cpp], bf16, name=f"p4h_{k}")
                nc.vector.tensor_copy(p4h[:], p4[k][:])
                for j in range(cpp):
                    nc.tensor.matmul(gps[:], p4h[:, :, j], wt4[k * cpp + j][:],
                                     start=(k == 0 and j == 0), stop=False)
            nc.tensor.matmul(gps[:], ones[:], bias_sb[:], start=False, stop=True)

            # ---- L2 normalize rows (fp32); Square+row-accum in one ACT op
            scr = pp.tile([bc, o_out], f32, name="scr")
            nrm2 = pp.tile([bc, 1], f32, name="nrm2")
            nc.scalar.activation(scr[:], gps[:], AF.Square, accum_out=nrm2[:])
            nrm = pp.tile([bc, 1], f32, name="nrm")
            nc.scalar.sqrt(nrm[:], nrm2[:])
            nmax = pp.tile([bc, 1], f32, name="nmax")
            nc.vector.tensor_scalar_max(nmax[:], nrm[:], NORM_EPS)
            rinv = pp.tile([bc, 1], f32, name="rinv")
            nc.vector.reciprocal(rinv[:], nmax[:])
            gn = pp.tile([bc, o_out], f32, name="gn")
            nc.scalar.mul(gn[:], gps[:], rinv[:])

            # local reciprocal distance: 1/max(||gn + eps||, DENO_EPS)  [bc, 1]
            eps_col = pp.tile([bc, 1], f32, name="eps_col")
            nc.vector.memset(eps_col, PD_EPS)
            dl2 = pp.tile([bc, 1], f32, name="dl2")
            nc.scalar.activation(scr[:], gn[:], AF.Square, bias=eps_col[:],
                                 accum_out=dl2[:])
            dl = pp.tile([bc, 1], f32, name="dl")
            nc.scalar.sqrt(dl[:], dl2[:])
            dlm = pp.tile([bc, 1], f32, name="dlm")
            nc.vector.tensor_scalar_max(dlm[:], dl[:], DENO_EPS)
            rdl = pp.tile([bc, 1], f32, name="rdl")
            nc.vector.reciprocal(rdl[:], dlm[:])

            # ---- AllGather payload: transposed rows [gnT | rdl] bf16 ----
            gnh = pp.tile([bc, o_out], bf16, name="gnh")
            nc.vector.tensor_copy(gnh[:], gn[:])
            for m in range(oc):
                psT = psp.tile([128, bc], bf16, name="psT", tag="rot")
                nc.tensor.transpose(psT[:], gnh[:, m * 128:(m + 1) * 128],
                                    identh[:bc, :bc])
                nc.vector.tensor_copy(ag_sb[:, m * bc:(m + 1) * bc], psT[:])
            nc.vector.tensor_copy(ag_sb[0:bc, oc * bc:oc * bc + 1], rdl[:])
            ag_in = dp.tile([128, agc], bf16, name="ag_in")
            ag_out = dp.tile([n_cores * 128, agc], bf16, name="ag_out",
                             addr_space="Shared")
            nc.sync.dma_start(ag_in[:], ag_sb[:])

            nc.gpsimd.collective_compute(
                "AllGather", AL.bypass,
                replica_groups=[list(range(n_cores))],
                ins=[ag_in.opt()], outs=[ag_out.opt()],
            )

            # gathered transposed blocks: gf_all[p, r, c] = ag_out[128r+p, c]
            gf_all = pp.tile([128, n_cores, agc], bf16, name="gf_all")
            nc.sync.dma_start(gf_all[:],
                              ag_out[:].rearrange("(r p) c -> p r c", p=128))

            # rdist row [1, b_full]: tiny PE transposes of each rank's rdl col
            psdr = psp.tile([1, b_full], bf16, name="psdr", tag="rot")
            for r in range(n_cores):
                nc.tensor.transpose(psdr[:, r * bc:(r + 1) * bc],
                                    gf_all[0:bc, r, oc * bc:oc * bc + 1],
                                    identh[:bc, :bc])
            rdrow = pp.tile([1, b_full], f32, name="rdrow")
            nc.scalar.copy(rdrow[:], psdr[:])

            # deno^-1 = min(rd_i, rd_j, 1/eps): broadcast rdist row, then
            # per-partition min with the local fp32 rdl
            dps = psgp.tile([bc, b_full], f32, name="dps")
            nc.tensor.matmul(dps[:], ones[:], rdrow[:], start=True, stop=True)
            den = pp.tile([bc, b_full], f32, name="den")
            nc.vector.tensor_scalar(den[:], dps[:], rdl[:], 1.0 / DENO_EPS,
                                    op0=AL.min, op1=AL.min)

            # sim block: [bc, b_full] = gn @ gf.T — lhsT is my own gnT slice,
            # rhs streams straight out of the gathered buffer (strided AP)
            sps = psgp.tile([bc, b_full], f32, name="sps")
            for m in range(oc):
                nc.tensor.matmul(sps[:], ag_sb[:, m * bc:(m + 1) * bc],
                                 gf_all[:, :, m * bc:(m + 1) * bc],
                                 start=(m == 0), stop=(m == oc - 1))

            outsb = pp.tile([bc, b_full], f32, name="outsb")
            nc.vector.tensor_mul(outsb[:], sps[:], den[:])
            nc.scalar.dma_start(out_d[:], outsb[:])

    nc.compile()
    return nc


_NC_CACHE = {}


def _get_nc():
    key = (B_FULL, C_IN, T_POOL, O_OUT, N_CORES)
    if key not in _NC_CACHE:
        _NC_CACHE[key] = build_kernel(*key)
    return _NC_CACHE[key]


def _run(features, W, bias, trace=False):
    from concourse.bass_utils import run_bass_kernel_spmd

    feats = np.ascontiguousarray(np.asarray(features, dtype=np.float32))
    w_np = np.ascontiguousarray(np.asarray(W, dtype=np.float32))
    bias_np = np.ascontiguousarray(
        np.asarray(bias, dtype=np.float32).reshape(1, O_OUT))
    bc = B_FULL // N_CORES

    nc = _get_nc()
    in_maps = [
        {"features": feats[r * bc:(r + 1) * bc], "w": w_np, "bias": bias_np}
        for r in range(N_CORES)
    ]
    res = run_bass_kernel_spmd(nc, in_maps, core_ids=list(range(N_CORES)),
                               trace=trace)
    out = np.concatenate([res.results[r]["out"] for r in range(N_CORES)], axis=0)
    return out, res.exec_time_ns


def kernel(features, W, bias):
    out, _ = _run(features, W, bias)
    return out


# revision 14
# speedup vs baseline: 1.0567x; 1.0567x over previous
"""Trainium2 Bass kernel for nn_ProjectionLayer: mean-pool + projection +
L2-normalize + cosine-sim matrix / pairwise-distance denominator.

Reference math (fp32):
    g = mean(features, axis=2) @ W.T + bias        # [b, out_c]
    g = g / max(||g||_row, 1e-12)                  # L2 normalize rows
    sim = g @ g.T                                  # [b, b]
    dist = ||g + 1e-6||_row                        # [b]
    out = sim / max(dist_i, dist_j, 1e-8)

Sharding: data-parallel over batch (64 rows per core, 8 cores). The
normalized rows are transposed on-chip, cast to bf16 and AllGather'd
together with a reciprocal-distance column (a [128, 257] payload,
66 KB/rank), so every core forms its [64, 512] block of the output with
no post-gather transposes: the sim matmul streams its rhs straight out
of the gathered buffer via a strided access pattern.
"""

import sys

if "/opt/trn_rl_repo" not in sys.path:
    sys.path.insert(0, "/opt/trn_rl_repo")

import numpy as np

# Problem shapes (hardcoded per contract)
B_FULL = 512     # batch
C_IN = 2048      # in channels (contraction dim of projection)
T_POOL = 196     # pooled (time) dim
O_OUT = 512      # out channels
N_CORES = 8

PD_EPS = 1e-6
NORM_EPS = 1e-12
DENO_EPS = 1e-8


def build_kernel(b_full, c_in, t_pool, o_out, n_cores, bg=2, cpp=8):
    """Emit the Bass module (SPMD program, identical on every core).

    cpp = channels per partition in the feature-pooling layout: partition p of
    chunk k holds channels c = span*k + cpp*p + j (j in [0, cpp)), which makes
    each DMA descriptor a cpp*t_pool*4-byte contiguous run (6.3 KB at cpp=8 —
    big descriptors keep the 16 SDMA engines at line rate). The channel
    interleave is undone for free by building W^T chunks from stride-cpp
    column slices of W.
    """
    import concourse.mybir as mybir
    import concourse.tile as tile
    from concourse import bacc
    from concourse.masks import make_identity

    f32 = mybir.dt.float32
    bf16 = mybir.dt.bfloat16
    AL = mybir.AluOpType
    AF = mybir.ActivationFunctionType

    bc = b_full // n_cores          # batch rows per core
    span = 128 * cpp                # channels per chunk
    nk = c_in // span               # channel chunks
    nbg = bc // bg                  # batch groups per chunk
    oc = o_out // 128               # out-channel chunks (for transposes)
    hb = 0                          # set below: batch-half split for projection
    agc = oc * bc + 1               # AG payload cols: 4*64 gnT cols + rdl
    hb = bc // 2
    assert bc % bg == 0 and c_in % span == 0 and o_out % 128 == 0

    nc = bacc.Bacc("TRN2", target_bir_lowering=False, debug=False,
                   enable_asserts=False, num_devices=n_cores)
    feat = nc.dram_tensor("features", [bc, c_in, t_pool], f32,
                          kind="ExternalInput").ap()
    w_in = nc.dram_tensor("w", [o_out, c_in], f32, kind="ExternalInput").ap()
    bias_in = nc.dram_tensor("bias", [1, o_out], f32, kind="ExternalInput").ap()
    out_d = nc.dram_tensor("out", [bc, b_full], f32, kind="ExternalOutput").ap()

    with tile.TileContext(nc) as tc:
        with (
            tc.tile_pool(name="const", bufs=1) as constp,
            tc.tile_pool(name="wload", bufs=1) as wlp,
            tc.tile_pool(name="wtp", bufs=1) as wtp,
            tc.tile_pool(name="featp", bufs=8) as fp,
            tc.tile_pool(name="lhsp", bufs=1) as lp,
            tc.tile_pool(name="postp", bufs=1) as pp,
            tc.tile_pool(name="psrot", bufs=2, space="PSUM") as psp,
            tc.tile_pool(name="psfix", bufs=1, space="PSUM") as psgp,
            tc.tile_pool(name="dram", bufs=1, space="DRAM") as dp,
        ):
            # ---- constants ----
            ident = constp.tile([128, 128], f32, name="ident")
            make_identity(nc, ident)
            identh = constp.tile([128, 128], bf16, name="identh")
            nc.vector.tensor_copy(identh[:], ident[:])
            ones = constp.tile([1, bc], f32, name="ones")
            nc.vector.memset(ones, 1.0)
            bias_sb = constp.tile([1, o_out], f32, name="bias_sb")
            nc.scalar.dma_start(bias_sb[:], bias_in[:])
            warm = constp.tile([1, 1], f32, name="warm")
            nc.vector.memset(warm, 1.0)
            nc.scalar.sqrt(warm[:], warm[:])
            # AG payload staging: gnT [128, 4*64] bf16 | rdl col (zero pad)
            ag_sb = pp.tile([128, agc], bf16, name="ag_sb")
            nc.gpsimd.memset(ag_sb, 0.0)

            p4 = [lp.tile([128, bc, cpp], f32, name=f"p4_{k}") for k in range(nk)]
            wl = []
            wt4 = [None] * (nk * cpp)
            gps = psgp.tile([bc, o_out], f32, name="gps")

            # ---- pooling (k outer -> incremental projection), fused with
            # W load / transpose mid-stream so both HWDGE rings start on
            # feature tiles immediately.
            idma = 0
            for k in range(nk):
                for ibg in range(nbg):
                    ft = fp.tile([128, bg, cpp, t_pool], f32, name="ft")
                    src = feat[ibg * bg:(ibg + 1) * bg,
                               k * span:(k + 1) * span, :].rearrange(
                                   "b (p j) t -> p b j t", j=cpp)
                    dma_eng = nc.sync if idma % 2 == 0 else nc.scalar
                    dma_eng.dma_start(ft[:], src)
                    nc.vector.reduce_sum(p4[k][:, ibg * bg:(ibg + 1) * bg, :],
                                         ft[:], axis=mybir.AxisListType.X)
                    if k == nk - 1 and ibg >= nbg - 4:
                        psdum = psp.tile([bc, o_out], f32, name="psdum",
                                         tag="rot")
                        for _ in range(3):
                            nc.tensor.matmul(psdum[:], ft[:, 0, 0, 0:bc],
                                             wl[0][:, 0:o_out],
                                             start=True, stop=True)
                    idma += 1

                    if k == 0 and ibg == 2:
                        # tiny warm-up AllGather: pre-touches the ncfw mesh
                        # path so the real gather pays less entry cost
                        agw_in = dp.tile([128, 16], bf16, name="agw_in")
                        agw_out = dp.tile([n_cores * 128, 16], bf16,
                                          name="agw_out", addr_space="Shared")
                        nc.scalar.dma_start(agw_in[:], ag_sb[:, 0:16])
                        nc.gpsimd.collective_compute(
                            "AllGather", AL.bypass,
                            replica_groups=[list(range(n_cores))],
                            ins=[agw_in.opt()], outs=[agw_out.opt()],
                        )

                    if k == 0 and ibg == 3:
                        # W loads: 4 MB total, interleaved into the ring queues
                        for l in range(oc):
                            wli = wlp.tile([128, c_in], f32, name=f"wl{l}")
                            eng = nc.sync if l % 2 == 0 else nc.scalar
                            eng.dma_start(wli[:], w_in[l * 128:(l + 1) * 128, :])
                            wl.append(wli)
                    if k == 0 and ibg == 8:
                        # W^T / t_pool, interleave-matched layout.
                        # wt4[kk*cpp+j] rows: partition p <-> channel
                        # c = span*kk + cpp*p + j. PE transposes (idle engine),
                        # PSUM->SBUF scale-copies on DVE (slack under DMA).
                        for kk in range(nk):
                            for j in range(cpp):
                                pswt = psp.tile([128, o_out], f32, name="pswt",
                                                tag="rot")
                                for l in range(oc):
                                    srcw = wl[l][:, kk * span:(kk + 1) * span
                                                 ].rearrange(
                                        "o (p j) -> o p j", j=cpp)[:, :, j]
                                    nc.tensor.transpose(
                                        pswt[:, l * 128:(l + 1) * 128],
                                        srcw, ident[:])
                                wtk = wtp.tile([128, o_out], bf16,
                                               name=f"wt{kk}_{j}")
                                nc.scalar.mul(wtk[:], pswt[:], 1.0 / t_pool)
                                wt4[kk * cpp + j] = wtk

                # incremental projection for this channel chunk: cast the
                # pooled block to bf16 (one DVE copy), single-pass bf16 matmuls
                p4h = lp.tile([128, bc, cpp], bf16, name=f"p4h_{k}")
                nc.vector.tensor_copy(p4h[:], p4[k][:])
                for j in range(cpp):
                    nc.tensor.matmul(gps[:], p4h[:, :, j], wt4[k * cpp + j][:],
                                     start=(k == 0 and j == 0), stop=False)
            nc.tensor.matmul(gps[:], ones[:], bias_sb[:], start=False, stop=True)

            # ---- L2 normalize rows (fp32); Square+row-accum in one ACT op
            scr = pp.tile([bc, o_out], f32, name="scr")
            nrm2 = pp.tile([bc, 1], f32, name="nrm2")
            nc.scalar.activation(scr[:], gps[:], AF.Square, accum_out=nrm2[:])
            nrm = pp.tile([bc, 1], f32, name="nrm")
            nc.scalar.sqrt(nrm[:], nrm2[:])
            nmax = pp.tile([bc, 1], f32, name="nmax")
            nc.vector.tensor_scalar_max(nmax[:], nrm[:], NORM_EPS)
            rinv = pp.tile([bc, 1], f32, name="rinv")
            nc.vector.reciprocal(rinv[:], nmax[:])
            gn = pp.tile([bc, o_out], f32, name="gn")
            nc.scalar.mul(gn[:], gps[:], rinv[:])

            # local reciprocal distance: 1/max(||gn + eps||, DENO_EPS)  [bc, 1]
            eps_col = pp.tile([bc, 1], f32, name="eps_col")
            nc.vector.memset(eps_col, PD_EPS)
            dl2 = pp.tile([bc, 1], f32, name="dl2")
            nc.scalar.activation(scr[:], gn[:], AF.Square, bias=eps_col[:],
                                 accum_out=dl2[:])
            dl = pp.tile([bc, 1], f32, name="dl")
            nc.scalar.sqrt(dl[:], dl2[:])
            dlm = pp.tile([bc, 1], f32, name="dlm")
            nc.vector.tensor_scalar_max(dlm[:], dl[:], DENO_EPS)
            rdl = pp.tile([bc, 1], f32, name="rdl")
            nc.vector.reciprocal(rdl[:], dlm[:])

            # ---- AllGather payload: transposed rows [gnT | rdl] bf16 ----
            gnh = pp.tile([bc, o_out], bf16, name="gnh")
            nc.vector.tensor_copy(gnh[:], gn[:])
            for m in range(oc):
                psT = psp.tile([128, bc], bf16, name="psT", tag="rot")
                nc.tensor.transpose(psT[:], gnh[:, m * 128:(m + 1) * 128],
                                    identh[:bc, :bc])
                nc.vector.tensor_copy(ag_sb[:, m * bc:(m + 1) * bc], psT[:])
            nc.vector.tensor_copy(ag_sb[0:bc, oc * bc:oc * bc + 1], rdl[:])
            ag_in = dp.tile([128, agc], bf16, name="ag_in")
            ag_out = dp.tile([n_cores * 128, agc], bf16, name="ag_out",
                             addr_space="Shared")
            nc.sync.dma_start(ag_in[:], ag_sb[:])

            nc.gpsimd.collective_compute(
                "AllGather", AL.bypass,
                replica_groups=[list(range(n_cores))],
                ins=[ag_in.opt()], outs=[ag_out.opt()],
            )

            # gathered transposed blocks: gf_all[p, r, c] = ag_out[128r+p, c]
            gf_all = pp.tile([128, n_cores, agc], bf16, name="gf_all")
            nc.sync.dma_start(gf_all[:],
                              ag_out[:].rearrange("(r p) c -> p r c", p=128))

            # rdist row [1, b_full]: tiny PE transposes of each rank's rdl col
            psdr = psp.tile([1, b_full], bf16, name="psdr", tag="rot")
            for r in range(n_cores):
                nc.tensor.transpose(psdr[:, r * bc:(r + 1) * bc],
                                    gf_all[0:bc, r, oc * bc:oc * bc + 1],
                                    identh[:bc, :bc])
            rdrow = pp.tile([1, b_full], f32, name="rdrow")
            nc.scalar.copy(rdrow[:], psdr[:])

            # deno^-1 = min(rd_i, rd_j, 1/eps): broadcast rdist row, then
            # per-partition min with the local fp32 rdl
            dps = psgp.tile([bc, b_full], f32, name="dps")
            nc.tensor.matmul(dps[:], ones[:], rdrow[:], start=True, stop=True)
            den = pp.tile([bc, b_full], f32, name="den")
            nc.vector.tensor_scalar(den[:], dps[:], rdl[:], 1.0 / DENO_EPS,
                                    op0=AL.min, op1=AL.min)

            # sim block: [bc, b_full] = gn @ gf.T — lhsT is my own gnT slice,
            # rhs streams straight out of the gathered buffer (strided AP)
            sps = psgp.tile([bc, b_full], f32, name="sps")
            for m in range(oc):
                nc.tensor.matmul(sps[:], ag_sb[:, m * bc:(m + 1) * bc],
                                 gf_all[:, :, m * bc:(m + 1) * bc],
                                 start=(m == 0), stop=(m == oc - 1))

            outsb = pp.tile([bc, b_full], f32, name="outsb")
            nc.vector.tensor_mul(outsb[:], sps[:], den[:])
            nc.scalar.dma_start(out_d[:], outsb[:])

    nc.compile()
    return nc


_NC_CACHE = {}


def _get_nc():
    key = (B_FULL, C_IN, T_POOL, O_OUT, N_CORES)
    if key not in _NC_CACHE:
        _NC_CACHE[key] = build_kernel(*key)
    return _NC_CACHE[key]


def _run(features, W, bias, trace=False):
    from concourse.bass_utils import run_bass_kernel_spmd

    feats = np.ascontiguousarray(np.asarray(features, dtype=np.float32))
    w_np = np.ascontiguousarray(np.asarray(W, dtype=np.float32))
    bias_np = np.ascontiguousarray(
        np.asarray(bias, dtype=np.float32).reshape(1, O_OUT))
    bc = B_FULL // N_CORES

    nc = _get_nc()
    in_maps = [
        {"features": feats[r * bc:(r + 1) * bc], "w": w_np, "bias": bias_np}
        for r in range(N_CORES)
    ]
    res = run_bass_kernel_spmd(nc, in_maps, core_ids=list(range(N_CORES)),
                               trace=trace)
    out = np.concatenate([res.results[r]["out"] for r in range(N_CORES)], axis=0)
    return out, res.exec_time_ns


def kernel(features, W, bias):
    out, _ = _run(features, W, bias)
    return out
